# revision 33
# baseline (speedup 1.0000x reference)
"""AttentionBlock (GroupNorm + 4-head self-attention + proj + residual) on 8 trn2 cores.

Sharding: data-parallel over batch. B=16 -> 2 batches per core. Weights replicated.

v2 dataflow per core (mixed precision, fp8 DoubleRow on the fat matmuls):
  x [2,256,1024] f32 -> GroupNorm stats via DVE/Pool row-sums + tiny selector
  matmuls; h = A*x+B applied on ACT -> fp8e4 [128, 2(chunk), N]
  qkv: fp8 DoubleRow matmuls (contract 256 in one pass); q,k -> bf16 tiles
  (bias folded on the psum->sbuf copy); V^T -> fp8e4 with per-head ones column
  S^T = k^T q per (head, m-chunk) in bf16, row-packed head pairs
  P = exp(S/8 - 3): half the chunks on ACT (-> fp8e4), half on DVE via a
  Schraudolph bit-trick (int8 write of e5m2 bits); the -3 shift cancels in
  softmax and keeps fp8e4 below TRN's 240 max
  AV: fp8 DoubleRow, lhsT = [V^T | ones] pairs -> rows 0..63 out, row 64 sums
  r = 1/sums (DVE, psum row) -> DRAM -> broadcast DMA [64, N] bf16
  ae = pav * r (DVE, psum->sbuf, bf16)
  proj bf16 per head (contract 64) + residual (po + pb + x) on DVE -> DRAM
"""
import numpy as np
from contextlib import ExitStack

import concourse.bass as bass
import concourse.bacc as bacc
import concourse.tile as tile
from concourse import mybir
from concourse import bass_utils

F32 = mybir.dt.float32
F32R = mybir.dt.float32r
BF16 = mybir.dt.bfloat16
F8E4 = mybir.dt.float8e4
F8E5 = mybir.dt.float8e5
I8 = mybir.dt.int8
AF = mybir.ActivationFunctionType
OP = mybir.AluOpType
DR = mybir.MatmulPerfMode.DoubleRow

B, C, H, W = 16, 256, 32, 32
N = H * W            # 1024
NH, D = 4, 64
G, GS = 32, 8        # groups, channels per group
EPS = 1e-5
NCORES = 8
BL = B // NCORES     # 2 batches per core
NCH = C // 128       # 2 channel chunks
NMC = N // 128       # 8 m-chunks
NPAIR = NMC // 2     # 4 DoubleRow m-chunk pairs
INV_GSZ = 1.0 / (GS * N)  # 1/8192
ATT_SCALE = 1.0 / np.sqrt(D)  # 0.125
VW = 72              # per-head V-block width in vt (pair stride must be 32-aligned)
SH = -3.0            # global softmax logit shift (cancels in normalize)
LOG2E = 1.4426950408889634
SCHRAU_MUL = 4.0 * LOG2E * ATT_SCALE          # S -> 4*log2(P)
SCHRAU_ADD = 4.0 * LOG2E * SH + 60.0 + 0.5    # +e5m2 bias, +0.5 round-half-up

_CACHE = {}


def _exp_on_act(hh, j):
    # alternate exp engine per row-packed head so ACT and DVE drain the S
    # psum ring in parallel; DVE carries recip/normalize too, so ACT takes
    # a bit more than half (10 of 16 tiles per phase)
    return hh == 0 or j == 3


def _build_nc(reps=1):
    import os
    cut = os.environ.get("KBISECT", "full")
    nc = bacc.Bacc()
    x = nc.declare_dram_parameter("x", [BL, C, N], F32R, isOutput=False)
    wqk_dr = nc.declare_dram_parameter("wqk_dr", [128, NCH, 2 * C], F8E4, isOutput=False)
    wv_dr = nc.declare_dram_parameter("wv_dr", [128, NCH, C], F8E4, isOutput=False)
    wpT4 = nc.declare_dram_parameter("wpT4", [D, NH * C], BF16, isOutput=False)
    # packed per-partition constants: gam0 gam1 bet0 bet1 qkb0..3 pb0 pb1
    cvec = nc.declare_dram_parameter("cvec", [128, 10], F32, isOutput=False)
    sel = nc.declare_dram_parameter("sel", [NCH, 128, G], F32R, isOutput=False)
    sel_exp = nc.declare_dram_parameter("sel_exp", [G, NCH, 128], F32, isOutput=False)
    out = nc.declare_dram_parameter("out", [BL, C, N], F32, isOutput=True)

    recip_dram = nc.dram_tensor("recip_dram", [BL * NH, N], BF16)

    def bcast_ap(dram_row_ap, parts):
        # DRAM row -> replicate across `parts` partitions (step-0 partition dim)
        return bass.AP(tensor=dram_row_ap.tensor, offset=dram_row_ap.offset,
                       ap=[[0, parts]] + [list(d) for d in dram_row_ap.ap])

    with tile.TileContext(nc) as tc, ExitStack() as ctx:
        if reps > 1:
            ctx.enter_context(tc.For_i(0, reps, 1, hint_engines=(
                mybir.EngineType.PE, mybir.EngineType.Activation,
                mybir.EngineType.DVE, mybir.EngineType.SP,
                mybir.EngineType.Pool)))
        const = ctx.enter_context(tc.tile_pool(name="const", bufs=1))
        xpool = ctx.enter_context(tc.tile_pool(name="xpool", bufs=2))
        hpool = ctx.enter_context(tc.tile_pool(name="hpool", bufs=2))
        tiny = ctx.enter_context(tc.tile_pool(name="tiny", bufs=1))
        abpool = ctx.enter_context(tc.tile_pool(name="abpool", bufs=2))
        stpool = ctx.enter_context(tc.tile_pool(name="stpool", bufs=2))
        scpool = ctx.enter_context(tc.tile_pool(name="scpool", bufs=2))
        qkpool = ctx.enter_context(tc.tile_pool(name="qkpool", bufs=2))
        vtpool = ctx.enter_context(tc.tile_pool(name="vtpool", bufs=2))
        ptpool = ctx.enter_context(tc.tile_pool(name="ptpool", bufs=2))
        rrpool = ctx.enter_context(tc.tile_pool(name="rrpool", bufs=4))
        bcpool = ctx.enter_context(tc.tile_pool(name="bcpool", bufs=2))
        aepool = ctx.enter_context(tc.tile_pool(name="aepool", bufs=2))
        outpool = ctx.enter_context(tc.tile_pool(name="outpool", bufs=2))
        psa = ctx.enter_context(tc.tile_pool(name="psa", bufs=2, space="PSUM"))
        psb = ctx.enter_context(tc.tile_pool(name="psb", bufs=2, space="PSUM"))

        # ---- x loads first (critical path), then constants ----
        xt = [[None] * NCH for _ in range(BL)]
        for b in range(BL):
            for c in range(NCH):
                t = xpool.tile([128, N], F32R, tag=f"xt{c}")
                nc.sync.dma_start(out=t, in_=x[b, c * 128:(c + 1) * 128, :])
                xt[b][c] = t
        sel_t = []
        for c in range(NCH):
            t = const.tile([128, G], F32R, tag=f"sel{c}")
            nc.sync.dma_start(out=t, in_=sel[c, :, :])
            sel_t.append(t)
        cvec_t = const.tile([128, 10], F32, tag="cvec")
        nc.sync.dma_start(out=cvec_t, in_=cvec[:, :])
        gam_t = [cvec_t[:, 0:1], cvec_t[:, 1:2]]
        bet_t = [cvec_t[:, 2:3], cvec_t[:, 3:4]]
        qkb_t = [cvec_t[:, 4 + j:5 + j] for j in range(4)]
        pb_t = [cvec_t[:, 8:9], cvec_t[:, 9:10]]
        selexp_t = const.tile([G, NCH * 128], F32, tag="selexp")
        nc.sync.dma_start(out=selexp_t, in_=sel_exp.rearrange("g c p -> g (c p)"))

        wqk_t = const.tile([128, NCH, 2 * C], F8E4, tag="wqk")
        nc.sync.dma_start(out=wqk_t, in_=wqk_dr[:, :, :])
        wv_t = const.tile([128, NCH, C], F8E4, tag="wv")
        nc.sync.dma_start(out=wv_t, in_=wv_dr[:, :, :])
        wp_t = const.tile([D, NH * C], BF16, tag="wpT")
        nc.sync.dma_start(out=wp_t, in_=wpT4[:, :])
        wpT_t = [wp_t[:, h * C:(h + 1) * C] for h in range(NH)]

        eps_t = tiny.tile([G, 1], F32, tag="epst")
        nc.vector.memset(eps_t, EPS)
        sh_t = tiny.tile([128, 1], F32, tag="sht")
        nc.vector.memset(sh_t, SH)

        # ---- PE warm-up: a few throwaway matmuls during the DMA wait keeps
        # the HAM clock gate from throttling the first real matmuls ----
        warm_ps = psb.tile([G, N], F32, tag="psb")
        for w in range(2):
            for nh2 in range(2):
                sl = slice(nh2 * 512, (nh2 + 1) * 512)
                nc.tensor.matmul(out=warm_ps[:, sl], lhsT=sel_t[0],
                                 rhs=xt[0][0][:, sl], start=True, stop=True)

        # ---- phase 1: groupnorm (stats via row-sums + tiny selector matmuls).
        # Ln/Exp batched across b to avoid ACT table thrash. ----
        ht = [None] * BL
        qk = [[None] * 4 for _ in range(BL)]
        vt = [None] * BL
        stbs = [None] * BL
        for b in range(BL):
            st = [None] * NCH
            for c in range(NCH):
                stc = stpool.tile([128, 2], F32R, tag=f"st{c}")
                st[c] = stc
                # sum(x) / sum(x^2) via ACT accumulate (tensor_tensor_reduce
                # faults real HW)
                with nc.allow_low_precision(reason="f32r groupnorm stat sums"):
                    scr0 = scpool.tile([128, N], BF16, tag="accs")
                    nc.scalar.activation(out=scr0, in_=xt[b][c].bitcast(F32),
                                         func=AF.Copy,
                                         accum_out=stc[:, 0:1])
                    scr = scpool.tile([128, N], BF16, tag="sqs")
                    nc.scalar.activation(out=scr, in_=xt[b][c].bitcast(F32),
                                         func=AF.Square,
                                         accum_out=stc[:, 1:2])
            g_ps = psa.tile([G, 2], F32, tag="psa")
            for c in range(NCH):
                nc.tensor.matmul(out=g_ps, lhsT=sel_t[c], rhs=st[c],
                                 start=(c == 0), stop=(c == NCH - 1))
            # stats cols: 2=mean 3=rstd 4=msq 5=m2 6=var 7=lnv
            stb = tiny.tile([G, 8], F32, tag=f"stats{b}")
            stbs[b] = stb
            nc.vector.tensor_scalar_mul(stb[:, 2:3], g_ps[:, 0:1], INV_GSZ)
            nc.vector.tensor_scalar_mul(stb[:, 4:5], g_ps[:, 1:2], INV_GSZ)
            nc.vector.tensor_tensor(out=stb[:, 5:6], in0=stb[:, 2:3], in1=stb[:, 2:3], op=OP.mult)
            nc.vector.tensor_tensor(out=stb[:, 6:7], in0=stb[:, 4:5], in1=stb[:, 5:6], op=OP.subtract)
        for b in range(BL):
            nc.scalar.activation(out=stbs[b][:, 7:8], in_=stbs[b][:, 6:7], func=AF.Ln, bias=eps_t)
        for b in range(BL):
            nc.scalar.activation(out=stbs[b][:, 3:4], in_=stbs[b][:, 7:8], func=AF.Exp, scale=-0.5)
        for b in range(BL):
            stb = stbs[b]
            hb = hpool.tile([128, NCH, N], F8E4, tag="h")
            ht[b] = hb
            for c in range(NCH):
                e_ps = psa.tile([128, 2], F32, tag="psa")
                nc.tensor.matmul(
                    out=e_ps,
                    lhsT=selexp_t[:, c * 128:(c + 1) * 128],
                    rhs=stb[:, 2:4],
                    start=True, stop=True)
                ab = abpool.tile([128, 3], F32, tag=f"ab{c}")
                nc.vector.tensor_tensor(out=ab[:, 0:1], in0=e_ps[:, 1:2], in1=gam_t[c], op=OP.mult)
                nc.vector.tensor_tensor(out=ab[:, 2:3], in0=e_ps[:, 0:1], in1=ab[:, 0:1], op=OP.mult)
                nc.vector.tensor_tensor(out=ab[:, 1:2], in0=bet_t[c], in1=ab[:, 2:3], op=OP.subtract)
                # h = A*x + B -> fp8e4, split across ACT/DVE
                if c == 0:
                    nc.scalar.activation(out=hb[:, c, :], in_=xt[b][c].bitcast(F32),
                                         func=AF.Identity,
                                         bias=ab[:, 1:2], scale=ab[:, 0:1])
                else:
                    nc.vector.tensor_scalar(
                        out=hb[:, c, :], in0=xt[b][c].bitcast(F32),
                        scalar1=ab[:, 0:1], scalar2=ab[:, 1:2],
                        op0=OP.mult, op1=OP.add)

        def dump(tiles):
            # debug bisection: copy arbitrary per-(b,c) tiles to out and stop
            for b in range(BL):
                for c in range(NCH):
                    ot = outpool.tile([128, N], F32, tag="ot")
                    src = tiles[b][c]
                    nc.vector.tensor_copy(out=ot[0:src.shape[0], 0:src.shape[-1]],
                                          in_=src)
                    nc.sync.dma_start(out=out[b, c * 128:(c + 1) * 128, :], in_=ot)

        if cut == "gn":
            dump([[ht[b][:, c, :] for c in range(NCH)] for b in range(BL)])
        # ---- phase 2: qkv (fp8 DoubleRow) ----
        for b in (range(BL) if cut != "gn" else []):
            for j in range(4):
                pj = psa.tile([128, N], F32, tag="psa")
                for nh2 in range(2):
                    sl = slice(nh2 * 512, (nh2 + 1) * 512)
                    nc.tensor.matmul(
                        out=pj[:, sl],
                        lhsT=wqk_t[:, :, j * 128:(j + 1) * 128],
                        rhs=ht[b][:, :, sl],
                        start=True, stop=True, perf_mode=DR)
                t = qkpool.tile([128, N], BF16, tag=f"qk{j}")
                if j % 2 == 0:
                    nc.scalar.activation(out=t, in_=pj, func=AF.Identity, bias=qkb_t[j])
                else:
                    nc.vector.tensor_scalar_add(t, pj, qkb_t[j])
                qk[b][j] = t

            vtb = vtpool.tile([128, NPAIR, 2, NH * VW], F8E4, tag="vt")
            vt[b] = vtb
            nc.gpsimd.memset(
                vtb.rearrange("p j i (h f) -> p j i h f", h=NH)[:, :, :, :, D:D + 1], 1.0)
            for mc in range(NMC):
                pv = psb.tile([128, N], F32, tag="psb")
                nc.tensor.matmul(
                    out=pv[:, 0:C],
                    lhsT=ht[b][:, :, mc * 128:(mc + 1) * 128],
                    rhs=wv_t,
                    start=True, stop=True, perf_mode=DR)
                vdst = vtb[:, mc // 2, mc % 2, :].rearrange("p (h f) -> p h f", h=NH)[:, :, 0:D]
                vsrc = pv[:, 0:C].rearrange("p (h f) -> p h f", h=NH)
                if mc % 2 == 0:
                    nc.scalar.activation(out=vdst, in_=vsrc, func=AF.Copy)
                else:
                    nc.vector.tensor_copy(out=vdst, in_=vsrc)

        # ---- phase 3: attention, (b, hp) interleaved ----
        pt = [[[None] * NPAIR for _ in range(NH)] for _ in range(BL)]
        ae = [[None] * NH for _ in range(BL)]

        def attn_phase(b, hp):
            qc = qk[b][hp]
            kc = qk[b][2 + hp]
            for j in range(NPAIR):
                for hh in range(2):
                    h = 2 * hp + hh
                    ptile = ptpool.tile([128, 2, N], F8E4 if _exp_on_act(hh, j) else F8E5,
                                        tag=f"pt{hh}{j}")
                    pt[b][h][j] = ptile
                for i in range(2):
                    mc = 2 * j + i
                    for hh in range(2):
                        h = 2 * hp + hh
                        rows = slice(hh * 64, hh * 64 + 64)
                        # alternate psum pools: effective 4-deep ring so the
                        # S matmuls never stall on the exp drain
                        pst = (psa if hh == 0 else psb).tile(
                            [128, N], F32, tag="psa" if hh == 0 else "psb")
                        for nh2 in range(2):
                            sl = slice(nh2 * 512, (nh2 + 1) * 512)
                            nc.tensor.matmul(
                                out=pst[:, sl],
                                lhsT=kc[rows, mc * 128:(mc + 1) * 128],
                                rhs=qc[rows, sl],
                                start=True, stop=True,
                                tile_position=(hh * 64, 0))
                        if _exp_on_act(hh, j):
                            nc.scalar.activation(out=pt[b][h][j][:, i, :], in_=pst,
                                                 func=AF.Exp, scale=ATT_SCALE, bias=sh_t)
                        else:
                            nc.vector.tensor_scalar(
                                out=pt[b][h][j].bitcast(I8)[:, i, :], in0=pst,
                                scalar1=SCHRAU_MUL, scalar2=SCHRAU_ADD,
                                op0=OP.mult, op1=OP.add)
            for hh in range(2):
                h = 2 * hp + hh
                pav = psb.tile([128, N], F32, tag="psb")
                for nh2 in range(2):
                    sl = slice(nh2 * 512, (nh2 + 1) * 512)
                    for j in range(NPAIR):
                        nc.tensor.matmul(
                            out=pav[0:D + 1, sl],
                            lhsT=vt[b][:, j, :, h * VW:h * VW + D + 1],
                            rhs=pt[b][h][j][:, :, sl],
                            start=(j == 0), stop=(j == NPAIR - 1),
                            perf_mode=DR)
                # drain psum immediately (frees the pav slot for the next
                # phase's S matmuls); normalize later on gpsimd from sbuf
                aeu = rrpool.tile([D + 1, N], BF16, tag=f"aeu{hh}")
                if hh == 0:
                    nc.scalar.activation(out=aeu, in_=pav[0:D + 1, :], func=AF.Copy)
                else:
                    nc.vector.tensor_copy(out=aeu, in_=pav[0:D + 1, :])
                with nc.allow_low_precision(reason="bf16 softmax recip"):
                    nc.vector.reciprocal(out=aeu[D:D + 1, :], in_=aeu[D:D + 1, :])
                nc.sync.dma_start(out=recip_dram[b * NH + h, :], in_=aeu[D:D + 1, :])
                bc = bcpool.tile([D, N], BF16, tag=f"bc{hh}")
                nc.sync.dma_start(out=bc, in_=bcast_ap(recip_dram[b * NH + h, :], D))
                aet = aepool.tile([D, N], BF16, tag=f"ae{h}")
                nc.gpsimd.tensor_tensor(out=aet, in0=aeu[0:D, :], in1=bc, op=OP.mult)
                ae[b][h] = aet

        def proj_phase(b):
            for c in range(NCH):
                po = psa.tile([128, N], F32, tag="psa")
                for h in range(NH):
                    for nh2 in range(2):
                        sl = slice(nh2 * 512, (nh2 + 1) * 512)
                        nc.tensor.matmul(
                            out=po[:, sl],
                            lhsT=wpT_t[h][:, c * 128:(c + 1) * 128],
                            rhs=ae[b][h][:, sl],
                            start=(h == 0), stop=(h == NH - 1))
                ot = outpool.tile([128, N], F32, tag="ot")
                for nh2 in range(2):
                    sl = slice(nh2 * 512, (nh2 + 1) * 512)
                    nc.vector.scalar_tensor_tensor(
                        out=ot[:, sl], in0=po[:, sl], scalar=pb_t[c],
                        in1=xt[b][c].bitcast(F32)[:, sl],
                        op0=OP.add, op1=OP.add)
                    nc.sync.dma_start(out=out[b, c * 128:(c + 1) * 128, sl],
                                      in_=ot[:, sl])

        if cut == "qkv":
            dump([[qk[b][c] for c in range(NCH)] for b in range(BL)])
        elif cut == "attn1":
            attn_phase(0, 0)
            attn_phase(1, 0)
            dump([[ae[b][c] for c in range(NCH)] for b in range(BL)])
        elif cut == "noproj":
            attn_phase(0, 0)
            attn_phase(1, 0)
            attn_phase(0, 1)
            attn_phase(1, 1)
            dump([[ae[b][2 + c] for c in range(NCH)] for b in range(BL)])
        elif cut == "full":
            attn_phase(0, 0)
            attn_phase(1, 0)
            attn_phase(0, 1)
            proj_phase(0)
            attn_phase(1, 1)
            proj_phase(1)

    nc.finalize()
    return nc


def _host_prep(x, gn_gamma, gn_beta, qkv_w, qkv_b, proj_w, proj_b):
    import ml_dtypes
    e4 = mybir.dt.np(F8E4)
    x = np.ascontiguousarray(np.asarray(x, dtype=np.float32)).reshape(B, C, N)
    qkv_w = np.asarray(qkv_w, dtype=np.float32)
    proj_w = np.asarray(proj_w, dtype=np.float32)
    qkv_b = np.asarray(qkv_b, dtype=np.float32)
    proj_b = np.asarray(proj_b, dtype=np.float32)

    def to_e4(a):
        return np.clip(a, -240.0, 240.0).astype(e4)

    # DoubleRow pair layouts: [p, chunk, out] with out = lhsT free dim
    wqk = qkv_w[:2 * C]                                   # [512, 256]
    wqk_dr = to_e4(np.ascontiguousarray(
        wqk.T.reshape(NCH, 128, 2 * C).transpose(1, 0, 2)))   # [128, 2, 512]
    wv = qkv_w[2 * C:]                                    # [256, 256]
    wv_dr = to_e4(np.ascontiguousarray(
        wv.T.reshape(NCH, 128, C).transpose(1, 0, 2)))        # [128, 2, 256]
    wpT = np.ascontiguousarray(proj_w.T)                  # [C', C]
    # [D, NH*C]: col h*C+c = proj_w[c, h*D+d]
    wpT4 = np.ascontiguousarray(
        wpT.reshape(NH, D, C).transpose(1, 0, 2).reshape(D, NH * C)
    ).astype(ml_dtypes.bfloat16)

    sel = np.zeros((NCH, 128, G), np.float32)
    for c in range(NCH):
        for p in range(128):
            sel[c, p, (c * 128 + p) // GS] = 1.0
    sel_exp = np.zeros((G, NCH, 128), np.float32)
    for c in range(NCH):
        for p in range(128):
            sel_exp[(c * 128 + p) // GS, c, p] = 1.0

    pb_eff = proj_b + proj_w @ qkv_b[2 * C:]   # fold v-bias into proj bias

    gam = np.asarray(gn_gamma, dtype=np.float32).reshape(NCH, 128)
    bet = np.asarray(gn_beta, dtype=np.float32).reshape(NCH, 128)
    cvec = np.stack([gam[0], gam[1], bet[0], bet[1],
                     qkv_b[0:128], qkv_b[128:256], qkv_b[256:384], qkv_b[384:512],
                     pb_eff[0:128].astype(np.float32), pb_eff[128:256].astype(np.float32)],
                    axis=1)
    shared = {
        "wqk_dr": wqk_dr, "wv_dr": wv_dr, "wpT4": wpT4,
        "cvec": np.ascontiguousarray(cvec),
        "sel": sel, "sel_exp": sel_exp,
    }
    in_maps = []
    for i in range(NCORES):
        m = dict(shared)
        m["x"] = np.ascontiguousarray(x[i * BL:(i + 1) * BL])
        in_maps.append(m)
    return in_maps


def _get_nc(reps=1):
    key = f"nc{reps}"
    if key not in _CACHE:
        _CACHE[key] = _build_nc(reps)
    return _CACHE[key]


def _pjrt_callable(nc):
    """Build the sharded jitted callable once (mirrors bass2jax.run_bass_via_pjrt)."""
    import jax
    from jax.sharding import Mesh, PartitionSpec, NamedSharding
    from jax.experimental.shard_map import shard_map
    from concourse import bass2jax, mybir as mb

    bass2jax.install_neuronx_cc_hook()
    partition_name = nc.partition_id_tensor.name if nc.partition_id_tensor else None
    in_names, out_names, out_avals, zero_outs = [], [], [], []
    for alloc in nc.m.functions[0].allocations:
        if not isinstance(alloc, mb.MemoryLocationSet):
            continue
        name = alloc.memorylocations[0].name
        if alloc.kind == "ExternalInput":
            if name != partition_name:
                in_names.append(name)
        elif alloc.kind == "ExternalOutput":
            out_names.append(name)
            out_avals.append(jax.core.ShapedArray(
                tuple(alloc.tensor_shape), mb.dt.np(alloc.dtype)))
            zero_outs.append(np.zeros(tuple(alloc.tensor_shape), mb.dt.np(alloc.dtype)))
    n_params = len(in_names)
    all_in_names = list(in_names) + list(out_names)
    if partition_name is not None:
        all_in_names.append(partition_name)

    def _body(*args):
        operands = list(args)
        if partition_name is not None:
            operands.append(bass2jax.partition_id_tensor())
        outs = bass2jax._bass_exec_p.bind(
            *operands,
            out_avals=tuple(out_avals),
            in_names=tuple(all_in_names),
            out_names=tuple(out_names),
            lowering_input_output_aliases=(),
            sim_require_finite=True,
            sim_require_nnan=True,
            nc=nc,
        )
        return tuple(outs)

    devices = jax.devices()[:NCORES]
    mesh = Mesh(np.asarray(devices), ("core",))
    nspec = n_params + len(out_names)
    sharded = jax.jit(
        shard_map(_body, mesh=mesh,
                  in_specs=(PartitionSpec("core"),) * nspec,
                  out_specs=(PartitionSpec("core"),) * len(out_names),
                  check_rep=False),
        keep_unused=True)
    return sharded, in_names, out_names, zero_outs, mesh


def run(inputs, iters=1, reps=1):
    """Run on HW via PJRT. Returns (out, dispatch wall times list)."""
    import jax, time
    from jax.sharding import NamedSharding, PartitionSpec
    nc = _get_nc(reps)
    in_maps = _host_prep(**inputs)
    ckey = f"callable{reps}"
    if ckey not in _CACHE:
        _CACHE[ckey] = _pjrt_callable(nc)
    sharded, in_names, out_names, zero_outs, mesh = _CACHE[ckey]

    concat_in = [np.concatenate([in_maps[c][n] for c in range(NCORES)], axis=0)
                 for n in in_names]
    concat_zeros = [np.zeros((NCORES * z.shape[0], *z.shape[1:]), z.dtype)
                    for z in zero_outs]
    sh = NamedSharding(mesh, PartitionSpec("core"))
    dev_in = [jax.device_put(a, sh) for a in concat_in]
    dev_zero = [jax.device_put(a, sh) for a in concat_zeros]

    out_arrs = jax.block_until_ready(sharded(*dev_in, *dev_zero))
    times = []
    for _ in range(max(0, iters - 1)):
        t0 = time.perf_counter()
        out_arrs2 = jax.block_until_ready(sharded(*dev_in, *dev_zero))
        t1 = time.perf_counter()
        times.append((t1 - t0) * 1e9)

    oi = out_names.index("out")
    out = np.asarray(out_arrs[oi]).reshape(B, C, H, W)
    return out, times


def kernel(**inputs):
    out, _ = run(inputs)
    return out
